# revision 18
# baseline (speedup 1.0000x reference)
"""Trainium2 Bass kernel for nn_Attention (B=2, S=2048, D=1024, H=16).

Sharding: tensor-parallel over heads. Each of the 8 cores owns 2 heads
(both batches): it computes q,k,v projections for its head columns, full
attention for its 4 (batch, head) pairs, and a partial output projection
(contraction over its 128 head-output columns). The host sums the 8
bf16 partials and adds b_proj.

Notes:
 - All matmuls bf16. fp8 (DoubleRow) was evaluated and rejected: with
   random weights the attention output is a near-uniform average over
   ~750 keys, so per-score quantization noise lands ~0.6x in the final
   output (fp8 q/k => 5% rel err, fp8 p => 2%). Only exact-math wins
   remain.
 - Both heads' K=64 score matmuls are issued as a pair occupying
   disjoint PE row-groups; they run concurrently.
 - Output partials are written bf16 (halves the out DMA).
 - x is loaded with one 3D DMA per (batch, query-tile) slab.
 - Side work (next batch's projections, out-projection rows) is pulled
   between exp and PV so the PE covers the ScalarE exp latency.
"""

import sys

sys.path.insert(0, "/opt/trn_rl_repo")

import numpy as np
import ml_dtypes

B, S, D, H, HD = 2, 2048, 1024, 16, 64
NCORES = 8
HPC = H // NCORES  # heads per core = 2
BS = B * S  # 4096
KB = S // 128  # key blocks per batch = 16
QT = 512  # query tile
NQT = S // QT  # query tiles per batch = 4
DC = D // 128  # contraction chunks = 8
PSHIFT = 1.0  # exp(s/8 + mask - PSHIFT); softmax-invariant shift

BF16 = ml_dtypes.bfloat16

_cache = {}


def _build(uniform_bias):
    import concourse.bass as bass
    import concourse.mybir as mybir
    import concourse.tile as tile
    from concourse import bacc
    from concourse.masks import make_identity

    fp32 = mybir.dt.float32
    bf16 = mybir.dt.bfloat16
    EXP = mybir.ActivationFunctionType.Exp

    nc = bacc.Bacc("TRN2", target_bir_lowering=False, debug=False,
                   num_devices=NCORES)

    xt_d = nc.dram_tensor("xt", [D, BS], bf16, kind="ExternalInput").ap()
    wq_d = nc.dram_tensor("wq", [D, 128], bf16, kind="ExternalInput").ap()
    wk_d = nc.dram_tensor("wk", [D, 128], bf16, kind="ExternalInput").ap()
    wv_d = nc.dram_tensor("wv", [D, 128], bf16, kind="ExternalInput").ap()
    bq_d = nc.dram_tensor("bq", [128, 1], fp32, kind="ExternalInput").ap()
    bk_d = nc.dram_tensor("bk", [128, 1], fp32, kind="ExternalInput").ap()
    bv_d = nc.dram_tensor("bv", [128, 1], fp32, kind="ExternalInput").ap()
    wp_d = nc.dram_tensor("wp", [128, D], bf16, kind="ExternalInput").ap()
    mk_d = nc.dram_tensor("maskt", [128, B * KB], fp32, kind="ExternalInput").ap()
    out_d = nc.dram_tensor("out", [BS, D], bf16, kind="ExternalOutput").ap()

    with tile.TileContext(nc) as tc:
        with (
            tc.tile_pool(name="const", bufs=1) as cpool,
            tc.tile_pool(name="xbf", bufs=64) as xbfpool,
            tc.tile_pool(name="qkv", bufs=2) as qkvpool,
            tc.tile_pool(name="vp", bufs=2 * HPC * KB) as vppool,
            tc.tile_pool(name="pt", bufs=8) as ptpool,
            tc.tile_pool(name="otn", bufs=2) as otnpool,
            tc.tile_pool(name="small", bufs=4) as smpool,
            tc.tile_pool(name="cout", bufs=3) as coutpool,
            tc.tile_pool(name="ps_a", bufs=2, space="PSUM") as ps_a,
            tc.tile_pool(name="ps_st", bufs=2, space="PSUM") as ps_st,
            tc.tile_pool(name="ps_ot", bufs=2, space="PSUM") as ps_ot,
        ):
            # ---- constants ----
            wq_sb = cpool.tile([128, DC, 128], bf16)
            wk_sb = cpool.tile([128, DC, 128], bf16)
            wv_sb = cpool.tile([128, DC, 128], bf16)
            for w_sb, w_d in ((wq_sb, wq_d), (wk_sb, wk_d), (wv_sb, wv_d)):
                nc.gpsimd.dma_start(w_sb[:], w_d.rearrange("(c p) m -> p c m", p=128))
            wp_sb = cpool.tile([128, D], bf16)
            nc.gpsimd.dma_start(wp_sb[:], wp_d)
            bq_sb = cpool.tile([128, 1], fp32)
            bk_sb = cpool.tile([128, 1], fp32)
            bv_sb = cpool.tile([128, 1], fp32)
            for b_sb, b_d in ((bq_sb, bq_d), (bk_sb, bk_d), (bv_sb, bv_d)):
                nc.gpsimd.dma_start(b_sb[:], b_d)
            mk_sb = cpool.tile([128, B * KB], fp32)
            nc.gpsimd.dma_start(mk_sb[:], mk_d)
            if uniform_bias is None:
                nc.vector.tensor_scalar_add(mk_sb[:], mk_sb[:], -PSHIFT)
            ident = cpool.tile([128, 128], bf16)
            make_identity(nc, ident[:])
            ubias_sb = cpool.tile([128, 1], fp32)
            if uniform_bias is not None:
                nc.gpsimd.memset(ubias_sb[:], uniform_bias)

            qkvs = {}
            vps = {}
            otns = {}

            def emit_vp(b, h, j):
                """One v' tile [128 keys, 64 v + ones] via PE transpose.
                Producer -> drain emitted atomically (no yield in between):
                a yield there lets other generators insert engine-queue ops
                between a psum ring slot's producer and its freeing consumer,
                which can deadlock the in-order queues."""
                vT = qkvs[b][2]
                vtr_ps = ps_a.tile([128, 64], bf16, tag="a", name="vtr_ps")
                nc.tensor.transpose(
                    vtr_ps[:],
                    vT[h * 64:(h + 1) * 64, j * 128:(j + 1) * 128],
                    ident[h * 64:(h + 1) * 64, h * 64:(h + 1) * 64])
                vp = vppool.tile([128, 65], bf16, tag="vp",
                                 name=f"vp_{b}_{h}_{j}")
                nc.vector.tensor_copy(vp[:, 0:64], vtr_ps[:])
                nc.gpsimd.memset(vp[:, 64:65], 1.0)
                vps[(b, h, j)] = vp
                yield 0

            def gen_a(b, head_only=False, tail_only=False):
                """Stage A for batch b, output-major order: all kT tiles,
                then vT (+v'), then qT(t0) [head]; qT(t1..t3) [tail].
                Yields 1 after each matmul, 0 after other units."""
                if not tail_only:
                    qT = qkvpool.tile([128, S], bf16, tag="qT", name=f"qT_{b}")
                    kT = qkvpool.tile([128, S], bf16, tag="kT", name=f"kT_{b}")
                    vT = qkvpool.tile([128, S], bf16, tag="vT", name=f"vT_{b}")
                    xbfs = []
                    for t in range(NQT):
                        xts = []
                        for c in range(DC):
                            xt = xbfpool.tile([128, QT], bf16, tag="xt",
                                              name="xt")
                            nc.sync.dma_start(
                                xt[:], xt_d[c * 128:(c + 1) * 128,
                                            b * S + t * QT:
                                            b * S + (t + 1) * QT])
                            xts.append(xt)
                        xbfs.append(xts)
                    qkvs[b] = (qT, kT, vT, xbfs)
                qT, kT, vT, xbfs = qkvs[b]

                def chain(dst, w_sb, b_sb, t, tg):
                    a_ps = ps_a.tile([128, QT], fp32, tag="a",
                                     name=f"a_ps_{tg}")
                    for c in range(DC):
                        nc.tensor.matmul(a_ps[:], w_sb[:, c, :],
                                         xbfs[t][c][:],
                                         start=(c == 0), stop=(c == DC - 1))
                        if c < DC - 1:
                            yield 1
                    # drain immediately after the last matmul (see emit_vp)
                    nc.vector.tensor_scalar_add(
                        dst[:, t * QT:(t + 1) * QT], a_ps[:], b_sb[:])
                    yield 1

                if not tail_only:
                    for t in range(NQT):
                        yield from chain(kT, wk_sb, bk_sb, t, "k")
                    for t in range(NQT):
                        yield from chain(vT, wv_sb, bv_sb, t, "v")
                        for h in range(HPC):
                            for j in range(4 * t, 4 * t + 4):
                                yield from emit_vp(b, h, j)
                    yield from chain(qT, wq_sb, bq_sb, 0, "q")
                if not head_only:
                    for t in range(1, NQT):
                        yield from chain(qT, wq_sb, bq_sb, t, "q")

            def gen_c(b, rows=None):
                """Stage C for batch b: partial out-projection, bf16 out."""
                otn = otns[b]
                for r in (range(S // 128) if rows is None else rows):
                    co = coutpool.tile([128, D], bf16, tag="co")
                    for n in range(D // QT):
                        c_ps = ps_a.tile([128, QT], fp32, tag="a", name="c_ps")
                        nc.tensor.matmul(c_ps[:],
                                         otn[:, r * 128:(r + 1) * 128],
                                         wp_sb[:, n * QT:(n + 1) * QT],
                                         start=True, stop=True)
                        # drain immediately (see emit_vp)
                        nc.vector.tensor_copy(co[:, n * QT:(n + 1) * QT],
                                              c_ps[:])
                        yield 1
                    nc.gpsimd.dma_start(
                        out_d[b * S + r * 128: b * S + (r + 1) * 128, :],
                        co[:])
                    yield 0

            def pull(side, extra=0):
                for e in side:
                    g, quota = e
                    for _ in range(quota + extra):
                        try:
                            next(g)
                        except StopIteration:
                            break

            def emit_attention(b, side, post_t_side=None, vp_provider=None):
                """Attention for batch b. `side` is a list of
                [generator, quota] entries; after each keyblock-pair,
                `quota` units are pulled from each live generator (fills
                the PE while ScalarE runs exp). post_t_side(t) may return
                an entry to append after query tile t's normalization."""
                qT, kT, vT = qkvs[b][:3]
                otn = otnpool.tile([128, S], bf16, tag="otn", name=f"otn_{b}")
                otns[b] = otn
                ot_ps = {}

                def norm(t, h):
                    ll = smpool.tile([1, QT], fp32, tag="ll")
                    nc.vector.tensor_copy(ll[:], ot_ps[(t, h)][64:65, :])
                    rc = smpool.tile([1, QT], fp32, tag="rc")
                    nc.vector.reciprocal_approx_fast(rc[:], ll[:])
                    bc = smpool.tile([64, QT], fp32, tag="bc")
                    nc.gpsimd.partition_broadcast(bc[:], rc[:])
                    if h == 0:
                        nc.vector.tensor_mul(
                            otn[0:64, t * QT:(t + 1) * QT],
                            ot_ps[(t, h)][0:64, :], bc[:])
                    else:
                        hb = smpool.tile([64, QT], bf16, tag="hb")
                        nc.vector.tensor_mul(hb[:], ot_ps[(t, h)][0:64, :],
                                             bc[:])
                        nc.gpsimd.dma_start(
                            otn[64:128, t * QT:(t + 1) * QT], hb[:])

                items = [(t, jj) for t in range(NQT) for jj in range(KB // 2)]
                pend = None  # (t, jj, pt2s) with PV not yet emitted

                def emit_pv(t, jj, pt2s):
                    while vp_provider is not None and \
                            (b, HPC - 1, 2 * jj + 1) not in vps:
                        next(vp_provider)
                    for h in range(HPC):
                        if jj == 0:
                            ot_ps[(t, h)] = ps_ot.tile(
                                [65, QT], fp32, tag="ot",
                                name=f"ot_ps_{b}_{t}_{h}")
                        nc.tensor.matmul(ot_ps[(t, h)][:],
                                         vps[(b, h, 2 * jj)][:],
                                         pt2s[h][:, 0, :],
                                         start=(jj == 0), stop=False)
                        nc.tensor.matmul(ot_ps[(t, h)][:],
                                         vps[(b, h, 2 * jj + 1)][:],
                                         pt2s[h][:, 1, :],
                                         start=False,
                                         stop=(jj == KB // 2 - 1))
                    if jj == KB // 2 - 1:
                        for h in range(HPC):
                            norm(t, h)
                        pull(side, extra=1)
                        if post_t_side is not None:
                            g = post_t_side(t)
                            if g is not None:
                                side.append(g)

                for (t, jj) in items:
                    j0, j1 = 2 * jj, 2 * jj + 1
                    st2s = [ps_st.tile([128, 2, QT], fp32, tag="st",
                                       name=f"st_{h}")
                            for h in range(HPC)]
                    # both heads' K=64 score matmuls issued as an
                    # atomic pair: they occupy disjoint PE row-groups
                    # (rows 0-63 / 64-127) and run concurrently.
                    for ji, jx in ((0, j0), (1, j1)):
                        for h in range(HPC):
                            hs = slice(h * 64, (h + 1) * 64)
                            nc.tensor.matmul(
                                st2s[h][:, ji, :],
                                kT[hs, jx * 128:(jx + 1) * 128],
                                qT[hs, t * QT:(t + 1) * QT],
                                start=True, stop=True)
                    pt2s = []
                    for h in range(HPC):
                        st2 = st2s[h]
                        pt2 = ptpool.tile([128, 2, QT], bf16, tag="pt",
                                          name=f"pt_{h}")
                        if uniform_bias is not None:
                            nc.scalar.activation(pt2[:], st2[:], EXP,
                                                 bias=ubias_sb[:],
                                                 scale=0.125)
                        else:
                            for ji, jx in ((0, j0), (1, j1)):
                                nc.scalar.activation(
                                    pt2[:, ji, :], st2[:, ji, :], EXP,
                                    bias=mk_sb[:, b * KB + jx:
                                               b * KB + jx + 1],
                                    scale=0.125)
                        pt2s.append(pt2)
                    # side work + previous pair's PV: the PE never waits
                    # on the exp it just issued
                    pull(side)
                    if pend is not None:
                        emit_pv(*pend)
                    pend = (t, jj, pt2s)
                emit_pv(*pend)

            def drain(gens):
                for g in gens:
                    for _ in g:
                        pass

            # batch 0: k/v/v'/q(t0) standalone; q(t1..3) + A(b1) fill the
            # exp-latency gaps of attn(b0); C(b0) rows released per-t
            drain([gen_a(0, head_only=True)])
            a0_tail = gen_a(0, tail_only=True)
            a1 = gen_a(1)
            side_b0 = [[a0_tail, 3], [a1, 6]]

            def post_t0(t):
                if t == 0:
                    return None
                return [gen_c(0, rows=range((t - 1) * NQT, t * NQT)), 1]

            emit_attention(0, side_b0, post_t_side=post_t0,
                           vp_provider=a0_tail)
            drain([a0_tail, a1])
            # attn(b1): remaining C(b0) + per-t-released C(b1)
            side_b1 = [[e[0], 2] for e in side_b0[2:]]
            side_b1.append([gen_c(0, rows=range(3 * NQT, S // 128)), 2])

            def post_t1(t):
                return [gen_c(1, rows=range(t * NQT, (t + 1) * NQT)), 2]

            emit_attention(1, side_b1, post_t_side=post_t1)
            drain([e[0] for e in side_b1])

    nc.compile()
    return nc


def _prep_inputs(x, attention_mask, w_attn, b_attn, w_proj):
    xT = np.ascontiguousarray(
        np.asarray(x, dtype=np.float32).reshape(BS, D).T).astype(BF16)
    maskt = np.ascontiguousarray(
        np.asarray(attention_mask, dtype=np.float32)
        .reshape(B, KB, 128).transpose(2, 0, 1).reshape(128, B * KB))
    w_attn = np.asarray(w_attn, dtype=np.float32)
    b_attn = np.asarray(b_attn, dtype=np.float32)
    w_proj = np.asarray(w_proj, dtype=np.float32)
    in_maps = []
    for c in range(NCORES):
        lo, hi = 2 * c * HD, (2 * c + 2) * HD
        in_maps.append({
            "xt": xT,
            "wq": np.ascontiguousarray(w_attn[:, lo:hi]).astype(BF16),
            "wk": np.ascontiguousarray(w_attn[:, D + lo: D + hi]).astype(BF16),
            "wv": np.ascontiguousarray(
                w_attn[:, 2 * D + lo: 2 * D + hi]).astype(BF16),
            "bq": np.ascontiguousarray(b_attn[lo:hi].reshape(128, 1)),
            "bk": np.ascontiguousarray(b_attn[D + lo: D + hi].reshape(128, 1)),
            "bv": np.ascontiguousarray(
                b_attn[2 * D + lo: 2 * D + hi].reshape(128, 1)),
            "wp": np.ascontiguousarray(w_proj[lo:hi, :]).astype(BF16),
            "maskt": maskt,
        })
    return in_maps


def _run(in_maps, trace=False, tmpdir=None):
    from concourse import bass_utils
    mk = in_maps[0]["maskt"]
    uniform = float(mk.flat[0]) if np.all(mk == mk.flat[0]) else None
    ub = None if uniform is None else uniform - PSHIFT
    key = ("nc", ub)
    if key not in _cache:
        _cache[key] = _build(ub)
    return bass_utils.run_bass_kernel_spmd(
        _cache[key], in_maps, core_ids=list(range(NCORES)),
        trace=trace, tmpdir=tmpdir)


def kernel(x, attention_mask, w_attn, b_attn, w_proj, b_proj):
    in_maps = _prep_inputs(x, attention_mask, w_attn, b_attn, w_proj)
    res = _run(in_maps)
    out = np.zeros((BS, D), dtype=np.float32)
    for c in range(NCORES):
        out += res.results[c]["out"].astype(np.float32)
    out += np.asarray(b_proj, dtype=np.float32)[None, :]
    return out.reshape(B, S, D)


# revision 19
# speedup vs baseline: 1.0180x; 1.0180x over previous
"""Trainium2 Bass kernel for nn_Attention (B=2, S=2048, D=1024, H=16).

Sharding: tensor-parallel over heads. Each of the 8 cores owns 2 heads
(both batches): it computes q,k,v projections for its head columns, full
attention for its 4 (batch, head) pairs, and a partial output projection
(contraction over its 128 head-output columns). The host sums the 8
bf16 partials and adds b_proj.

Notes:
 - All matmuls bf16. fp8 (DoubleRow) was evaluated and rejected: with
   random weights the attention output is a near-uniform average over
   ~750 keys, so per-score quantization noise lands ~0.6x in the final
   output (fp8 q/k => 5% rel err, fp8 p => 2%). Only exact-math wins
   remain.
 - Both heads' K=64 score matmuls are issued as a pair occupying
   disjoint PE row-groups; they run concurrently.
 - Output partials are written bf16 (halves the out DMA).
 - x is loaded with one 3D DMA per (batch, query-tile) slab.
 - Side work (next batch's projections, out-projection rows) is pulled
   between exp and PV so the PE covers the ScalarE exp latency.
"""

import sys

sys.path.insert(0, "/opt/trn_rl_repo")

import numpy as np
import ml_dtypes

B, S, D, H, HD = 2, 2048, 1024, 16, 64
NCORES = 8
HPC = H // NCORES  # heads per core = 2
BS = B * S  # 4096
KB = S // 128  # key blocks per batch = 16
QT = 512  # query tile
NQT = S // QT  # query tiles per batch = 4
DC = D // 128  # contraction chunks = 8
PSHIFT = 1.0  # exp(s/8 + mask - PSHIFT); softmax-invariant shift

BF16 = ml_dtypes.bfloat16

_cache = {}


def _build(uniform_bias):
    import concourse.bass as bass
    import concourse.mybir as mybir
    import concourse.tile as tile
    from concourse import bacc
    from concourse.masks import make_identity

    fp32 = mybir.dt.float32
    bf16 = mybir.dt.bfloat16
    EXP = mybir.ActivationFunctionType.Exp

    nc = bacc.Bacc("TRN2", target_bir_lowering=False, debug=False,
                   num_devices=NCORES)

    xt_d = nc.dram_tensor("xt", [D, BS], bf16, kind="ExternalInput").ap()
    wq_d = nc.dram_tensor("wq", [D, 128], bf16, kind="ExternalInput").ap()
    wk_d = nc.dram_tensor("wk", [D, 128], bf16, kind="ExternalInput").ap()
    wv_d = nc.dram_tensor("wv", [D, 128], bf16, kind="ExternalInput").ap()
    bq_d = nc.dram_tensor("bq", [128, 1], fp32, kind="ExternalInput").ap()
    bk_d = nc.dram_tensor("bk", [128, 1], fp32, kind="ExternalInput").ap()
    bv_d = nc.dram_tensor("bv", [128, 1], fp32, kind="ExternalInput").ap()
    wp_d = nc.dram_tensor("wp", [128, D], bf16, kind="ExternalInput").ap()
    mk_d = nc.dram_tensor("maskt", [128, B * KB], fp32, kind="ExternalInput").ap()
    out_d = nc.dram_tensor("out", [BS, D], bf16, kind="ExternalOutput").ap()

    with tile.TileContext(nc) as tc:
        with (
            tc.tile_pool(name="const", bufs=1) as cpool,
            tc.tile_pool(name="xbf", bufs=64) as xbfpool,
            tc.tile_pool(name="qkv", bufs=2) as qkvpool,
            tc.tile_pool(name="vp", bufs=2 * HPC * KB) as vppool,
            tc.tile_pool(name="pt", bufs=8) as ptpool,
            tc.tile_pool(name="otn", bufs=2) as otnpool,
            tc.tile_pool(name="small", bufs=4) as smpool,
            tc.tile_pool(name="cout", bufs=3) as coutpool,
            tc.tile_pool(name="ps_a", bufs=2, space="PSUM") as ps_a,
            tc.tile_pool(name="ps_st", bufs=2, space="PSUM") as ps_st,
            tc.tile_pool(name="ps_ot", bufs=2, space="PSUM") as ps_ot,
        ):
            # ---- constants ----
            wq_sb = cpool.tile([128, DC, 128], bf16)
            wk_sb = cpool.tile([128, DC, 128], bf16)
            wv_sb = cpool.tile([128, DC, 128], bf16)
            for w_sb, w_d in ((wq_sb, wq_d), (wk_sb, wk_d), (wv_sb, wv_d)):
                nc.gpsimd.dma_start(w_sb[:], w_d.rearrange("(c p) m -> p c m", p=128))
            wp_sb = cpool.tile([128, D], bf16)
            nc.gpsimd.dma_start(wp_sb[:], wp_d)
            bq_sb = cpool.tile([128, 1], fp32)
            bk_sb = cpool.tile([128, 1], fp32)
            bv_sb = cpool.tile([128, 1], fp32)
            for b_sb, b_d in ((bq_sb, bq_d), (bk_sb, bk_d), (bv_sb, bv_d)):
                nc.gpsimd.dma_start(b_sb[:], b_d)
            mk_sb = cpool.tile([128, B * KB], fp32)
            nc.gpsimd.dma_start(mk_sb[:], mk_d)
            if uniform_bias is None:
                nc.vector.tensor_scalar_add(mk_sb[:], mk_sb[:], -PSHIFT)
            ident = cpool.tile([128, 128], bf16)
            make_identity(nc, ident[:])
            ubias_sb = cpool.tile([128, 1], fp32)
            if uniform_bias is not None:
                nc.gpsimd.memset(ubias_sb[:], uniform_bias)

            qkvs = {}
            vps = {}
            otns = {}

            def emit_vp(b, h, j):
                """One v' tile [128 keys, 64 v + ones] via PE transpose.
                Producer -> drain emitted atomically (no yield in between):
                a yield there lets other generators insert engine-queue ops
                between a psum ring slot's producer and its freeing consumer,
                which can deadlock the in-order queues."""
                vT = qkvs[b][2]
                vtr_ps = ps_a.tile([128, 64], bf16, tag="a", name="vtr_ps")
                nc.tensor.transpose(
                    vtr_ps[:],
                    vT[h * 64:(h + 1) * 64, j * 128:(j + 1) * 128],
                    ident[h * 64:(h + 1) * 64, h * 64:(h + 1) * 64])
                vp = vppool.tile([128, 65], bf16, tag="vp",
                                 name=f"vp_{b}_{h}_{j}")
                nc.vector.tensor_copy(vp[:, 0:64], vtr_ps[:])
                nc.gpsimd.memset(vp[:, 64:65], 1.0)
                vps[(b, h, j)] = vp
                yield 0

            def gen_a(b, head_only=False, tail_only=False):
                """Stage A for batch b, output-major order: all kT tiles,
                then vT (+v'), then qT(t0) [head]; qT(t1..t3) [tail].
                Yields 1 after each matmul, 0 after other units."""
                if not tail_only:
                    qT = qkvpool.tile([128, S], bf16, tag="qT", name=f"qT_{b}")
                    kT = qkvpool.tile([128, S], bf16, tag="kT", name=f"kT_{b}")
                    vT = qkvpool.tile([128, S], bf16, tag="vT", name=f"vT_{b}")
                    xbfs = []
                    for t in range(NQT):
                        xts = []
                        for c in range(DC):
                            xt = xbfpool.tile([128, QT], bf16, tag="xt",
                                              name="xt")
                            nc.sync.dma_start(
                                xt[:], xt_d[c * 128:(c + 1) * 128,
                                            b * S + t * QT:
                                            b * S + (t + 1) * QT])
                            xts.append(xt)
                        xbfs.append(xts)
                    qkvs[b] = (qT, kT, vT, xbfs)
                qT, kT, vT, xbfs = qkvs[b]

                def chain(dst, w_sb, b_sb, t, tg):
                    a_ps = ps_a.tile([128, QT], fp32, tag="a",
                                     name=f"a_ps_{tg}")
                    for c in range(DC):
                        nc.tensor.matmul(a_ps[:], w_sb[:, c, :],
                                         xbfs[t][c][:],
                                         start=(c == 0), stop=(c == DC - 1))
                        if c < DC - 1:
                            yield 1
                    # drain immediately after the last matmul (see emit_vp)
                    nc.vector.tensor_scalar_add(
                        dst[:, t * QT:(t + 1) * QT], a_ps[:], b_sb[:])
                    yield 1

                def vvp(t):
                    yield from chain(vT, wv_sb, bv_sb, t, "v")
                    for j in range(4 * t, 4 * t + 4):
                        for h in range(HPC):
                            yield from emit_vp(b, h, j)

                if not tail_only:
                    for t in range(NQT):
                        yield from chain(kT, wk_sb, bk_sb, t, "k")
                    yield from vvp(0)
                    yield from chain(qT, wq_sb, bq_sb, 0, "q")
                if not head_only:
                    for t in range(1, NQT):
                        yield from vvp(t)
                        yield from chain(qT, wq_sb, bq_sb, t, "q")

            def gen_c(b, rows=None):
                """Stage C for batch b: partial out-projection, bf16 out."""
                otn = otns[b]
                for r in (range(S // 128) if rows is None else rows):
                    co = coutpool.tile([128, D], bf16, tag="co")
                    for n in range(D // QT):
                        c_ps = ps_a.tile([128, QT], fp32, tag="a", name="c_ps")
                        nc.tensor.matmul(c_ps[:],
                                         otn[:, r * 128:(r + 1) * 128],
                                         wp_sb[:, n * QT:(n + 1) * QT],
                                         start=True, stop=True)
                        # drain immediately (see emit_vp)
                        nc.vector.tensor_copy(co[:, n * QT:(n + 1) * QT],
                                              c_ps[:])
                        yield 1
                    nc.gpsimd.dma_start(
                        out_d[b * S + r * 128: b * S + (r + 1) * 128, :],
                        co[:])
                    yield 0

            def pull(side, extra=0):
                for e in side:
                    g, quota = e
                    for _ in range(quota + extra):
                        try:
                            next(g)
                        except StopIteration:
                            break

            def emit_attention(b, side, post_t_side=None, vp_provider=None):
                """Attention for batch b. `side` is a list of
                [generator, quota] entries; after each keyblock-pair,
                `quota` units are pulled from each live generator (fills
                the PE while ScalarE runs exp). post_t_side(t) may return
                an entry to append after query tile t's normalization."""
                qT, kT, vT = qkvs[b][:3]
                otn = otnpool.tile([128, S], bf16, tag="otn", name=f"otn_{b}")
                otns[b] = otn
                ot_ps = {}

                def norm(t, h):
                    ll = smpool.tile([1, QT], fp32, tag="ll")
                    nc.vector.tensor_copy(ll[:], ot_ps[(t, h)][64:65, :])
                    rc = smpool.tile([1, QT], fp32, tag="rc")
                    nc.vector.reciprocal_approx_fast(rc[:], ll[:])
                    bc = smpool.tile([64, QT], fp32, tag="bc")
                    nc.gpsimd.partition_broadcast(bc[:], rc[:])
                    if h == 0:
                        nc.vector.tensor_mul(
                            otn[0:64, t * QT:(t + 1) * QT],
                            ot_ps[(t, h)][0:64, :], bc[:])
                    else:
                        hb = smpool.tile([64, QT], bf16, tag="hb")
                        nc.vector.tensor_mul(hb[:], ot_ps[(t, h)][0:64, :],
                                             bc[:])
                        nc.gpsimd.dma_start(
                            otn[64:128, t * QT:(t + 1) * QT], hb[:])

                items = [(t, jj) for t in range(NQT) for jj in range(KB // 2)]
                pend = None  # (t, jj, pt2s) with PV not yet emitted

                def emit_pv(t, jj, pt2s):
                    while vp_provider is not None and \
                            (b, HPC - 1, 2 * jj + 1) not in vps:
                        next(vp_provider)
                    for h in range(HPC):
                        if jj == 0:
                            ot_ps[(t, h)] = ps_ot.tile(
                                [65, QT], fp32, tag="ot",
                                name=f"ot_ps_{b}_{t}_{h}")
                        nc.tensor.matmul(ot_ps[(t, h)][:],
                                         vps[(b, h, 2 * jj)][:],
                                         pt2s[h][:, 0, :],
                                         start=(jj == 0), stop=False)
                        nc.tensor.matmul(ot_ps[(t, h)][:],
                                         vps[(b, h, 2 * jj + 1)][:],
                                         pt2s[h][:, 1, :],
                                         start=False,
                                         stop=(jj == KB // 2 - 1))
                    if jj == KB // 2 - 1:
                        for h in range(HPC):
                            norm(t, h)
                        pull(side, extra=1)
                        if post_t_side is not None:
                            g = post_t_side(t)
                            if g is not None:
                                side.append(g)

                for (t, jj) in items:
                    j0, j1 = 2 * jj, 2 * jj + 1
                    st2s = [ps_st.tile([128, 2, QT], fp32, tag="st",
                                       name=f"st_{h}")
                            for h in range(HPC)]
                    # both heads' K=64 score matmuls issued as an
                    # atomic pair: they occupy disjoint PE row-groups
                    # (rows 0-63 / 64-127) and run concurrently.
                    for ji, jx in ((0, j0), (1, j1)):
                        for h in range(HPC):
                            hs = slice(h * 64, (h + 1) * 64)
                            nc.tensor.matmul(
                                st2s[h][:, ji, :],
                                kT[hs, jx * 128:(jx + 1) * 128],
                                qT[hs, t * QT:(t + 1) * QT],
                                start=True, stop=True)
                    pt2s = []
                    for h in range(HPC):
                        st2 = st2s[h]
                        pt2 = ptpool.tile([128, 2, QT], bf16, tag="pt",
                                          name=f"pt_{h}")
                        if uniform_bias is not None:
                            nc.scalar.activation(pt2[:], st2[:], EXP,
                                                 bias=ubias_sb[:],
                                                 scale=0.125)
                        else:
                            for ji, jx in ((0, j0), (1, j1)):
                                nc.scalar.activation(
                                    pt2[:, ji, :], st2[:, ji, :], EXP,
                                    bias=mk_sb[:, b * KB + jx:
                                               b * KB + jx + 1],
                                    scale=0.125)
                        pt2s.append(pt2)
                    # side work + previous pair's PV: the PE never waits
                    # on the exp it just issued
                    pull(side)
                    if pend is not None:
                        emit_pv(*pend)
                    pend = (t, jj, pt2s)
                emit_pv(*pend)

            def drain(gens):
                for g in gens:
                    for _ in g:
                        pass

            # batch 0: k/v/v'/q(t0) standalone; q(t1..3) + A(b1) fill the
            # exp-latency gaps of attn(b0); C(b0) rows released per-t
            drain([gen_a(0, head_only=True)])
            a0_tail = gen_a(0, tail_only=True)
            a1 = gen_a(1)
            side_b0 = [[a0_tail, 3], [a1, 6]]

            def post_t0(t):
                if t == 0:
                    return None
                return [gen_c(0, rows=range((t - 1) * NQT, t * NQT)), 1]

            emit_attention(0, side_b0, post_t_side=post_t0,
                           vp_provider=a0_tail)
            drain([a0_tail, a1])
            # attn(b1): remaining C(b0) + per-t-released C(b1)
            side_b1 = [[e[0], 2] for e in side_b0[2:]]
            side_b1.append([gen_c(0, rows=range(3 * NQT, S // 128)), 2])

            def post_t1(t):
                return [gen_c(1, rows=range(t * NQT, (t + 1) * NQT)), 2]

            emit_attention(1, side_b1, post_t_side=post_t1)
            drain([e[0] for e in side_b1])

    nc.compile()
    return nc


def _prep_inputs(x, attention_mask, w_attn, b_attn, w_proj):
    xT = np.ascontiguousarray(
        np.asarray(x, dtype=np.float32).reshape(BS, D).T).astype(BF16)
    maskt = np.ascontiguousarray(
        np.asarray(attention_mask, dtype=np.float32)
        .reshape(B, KB, 128).transpose(2, 0, 1).reshape(128, B * KB))
    w_attn = np.asarray(w_attn, dtype=np.float32)
    b_attn = np.asarray(b_attn, dtype=np.float32)
    w_proj = np.asarray(w_proj, dtype=np.float32)
    in_maps = []
    for c in range(NCORES):
        lo, hi = 2 * c * HD, (2 * c + 2) * HD
        in_maps.append({
            "xt": xT,
            "wq": np.ascontiguousarray(w_attn[:, lo:hi]).astype(BF16),
            "wk": np.ascontiguousarray(w_attn[:, D + lo: D + hi]).astype(BF16),
            "wv": np.ascontiguousarray(
                w_attn[:, 2 * D + lo: 2 * D + hi]).astype(BF16),
            "bq": np.ascontiguousarray(b_attn[lo:hi].reshape(128, 1)),
            "bk": np.ascontiguousarray(b_attn[D + lo: D + hi].reshape(128, 1)),
            "bv": np.ascontiguousarray(
                b_attn[2 * D + lo: 2 * D + hi].reshape(128, 1)),
            "wp": np.ascontiguousarray(w_proj[lo:hi, :]).astype(BF16),
            "maskt": maskt,
        })
    return in_maps


def _run(in_maps, trace=False, tmpdir=None):
    from concourse import bass_utils
    mk = in_maps[0]["maskt"]
    uniform = float(mk.flat[0]) if np.all(mk == mk.flat[0]) else None
    ub = None if uniform is None else uniform - PSHIFT
    key = ("nc", ub)
    if key not in _cache:
        _cache[key] = _build(ub)
    return bass_utils.run_bass_kernel_spmd(
        _cache[key], in_maps, core_ids=list(range(NCORES)),
        trace=trace, tmpdir=tmpdir)


def kernel(x, attention_mask, w_attn, b_attn, w_proj, b_proj):
    in_maps = _prep_inputs(x, attention_mask, w_attn, b_attn, w_proj)
    res = _run(in_maps)
    out = np.zeros((BS, D), dtype=np.float32)
    for c in range(NCORES):
        out += res.results[c]["out"].astype(np.float32)
    out += np.asarray(b_proj, dtype=np.float32)[None, :]
    return out.reshape(B, S, D)


# revision 30
# speedup vs baseline: 1.2326x; 1.2108x over previous
"""Trainium2 Bass kernel for nn_Attention (B=2, S=2048, D=1024, H=16).

Sharding: tensor-parallel over heads. Each of the 8 cores owns 2 heads
(both batches): it computes q,k,v projections for its head columns, full
attention for its 4 (batch, head) pairs, and a partial output projection
(contraction over its 128 head-output columns). The host sums the 8
bf16 partials and adds b_proj.

Notes:
 - All matmuls bf16. fp8 (DoubleRow) was evaluated and rejected: with
   random weights the attention output is a near-uniform average over
   ~750 keys, so per-score quantization noise lands ~0.6x in the final
   output (fp8 q/k => 5% rel err, fp8 p => 2%). Only exact-math wins
   remain.
 - Both heads' K=64 score matmuls are issued as a pair occupying
   disjoint PE row-groups; they run concurrently.
 - Output partials are written bf16 (halves the out DMA).
 - x is loaded with one 3D DMA per (batch, query-tile) slab.
 - Side work (next batch's projections, out-projection rows) is pulled
   between exp and PV so the PE covers the ScalarE exp latency.
"""

import sys

sys.path.insert(0, "/opt/trn_rl_repo")

import numpy as np
import ml_dtypes

B, S, D, H, HD = 2, 2048, 1024, 16, 64
NCORES = 8
HPC = H // NCORES  # heads per core = 2
BS = B * S  # 4096
KB = S // 128  # key blocks per batch = 16
QT = 512  # query tile
NQT = S // QT  # query tiles per batch = 4
DC = D // 128  # contraction chunks = 8
PSHIFT = 1.0  # exp(s/8 + mask - PSHIFT); softmax-invariant shift

BF16 = ml_dtypes.bfloat16

_cache = {}


def _build(uniform_bias):
    import concourse.bass as bass
    import concourse.mybir as mybir
    import concourse.tile as tile
    from concourse import bacc
    from concourse.masks import make_identity

    fp32 = mybir.dt.float32
    bf16 = mybir.dt.bfloat16
    EXP = mybir.ActivationFunctionType.Exp

    nc = bacc.Bacc("TRN2", target_bir_lowering=False, debug=False,
                   num_devices=NCORES)

    xt_d = nc.dram_tensor("xt", [D, BS], bf16, kind="ExternalInput").ap()
    wq_d = nc.dram_tensor("wq", [D, 128], bf16, kind="ExternalInput").ap()
    wk_d = nc.dram_tensor("wk", [D, 128], bf16, kind="ExternalInput").ap()
    wv_d = nc.dram_tensor("wv", [D, 128], bf16, kind="ExternalInput").ap()
    bq_d = nc.dram_tensor("bq", [128, 1], fp32, kind="ExternalInput").ap()
    bk_d = nc.dram_tensor("bk", [128, 1], fp32, kind="ExternalInput").ap()
    bv_d = nc.dram_tensor("bv", [128, 1], fp32, kind="ExternalInput").ap()
    wp_d = nc.dram_tensor("wp", [128, D], bf16, kind="ExternalInput").ap()
    mk_d = nc.dram_tensor("maskt", [128, B * KB], fp32, kind="ExternalInput").ap()
    out_d = nc.dram_tensor("out", [BS, D], bf16, kind="ExternalOutput").ap()

    with tile.TileContext(nc) as tc:
        with (
            tc.tile_pool(name="const", bufs=1) as cpool,
            tc.tile_pool(name="xbf", bufs=64) as xbfpool,
            tc.tile_pool(name="qkv", bufs=2) as qkvpool,
            tc.tile_pool(name="vp", bufs=2 * HPC * KB) as vppool,
            tc.tile_pool(name="pt", bufs=8) as ptpool,
            tc.tile_pool(name="otn", bufs=2) as otnpool,
            tc.tile_pool(name="small", bufs=4) as smpool,
            tc.tile_pool(name="cout", bufs=3) as coutpool,
            tc.tile_pool(name="ps_a", bufs=2, space="PSUM") as ps_a,
            tc.tile_pool(name="ps_st", bufs=2, space="PSUM") as ps_st,
            tc.tile_pool(name="ps_ot", bufs=2, space="PSUM") as ps_ot,
        ):
            # ---- constants ----
            wq_sb = cpool.tile([128, DC, 128], bf16)
            wk_sb = cpool.tile([128, DC, 128], bf16)
            wv_sb = cpool.tile([128, DC, 128], bf16)
            for w_sb, w_d in ((wq_sb, wq_d), (wk_sb, wk_d), (wv_sb, wv_d)):
                nc.gpsimd.dma_start(w_sb[:], w_d.rearrange("(c p) m -> p c m", p=128))
            wp_sb = cpool.tile([128, D], bf16)
            nc.gpsimd.dma_start(wp_sb[:], wp_d)
            bq_sb = cpool.tile([128, 1], fp32)
            bk_sb = cpool.tile([128, 1], fp32)
            bv_sb = cpool.tile([128, 1], fp32)
            for b_sb, b_d in ((bq_sb, bq_d), (bk_sb, bk_d), (bv_sb, bv_d)):
                nc.gpsimd.dma_start(b_sb[:], b_d)
            mk_sb = cpool.tile([128, B * KB], fp32)
            nc.gpsimd.dma_start(mk_sb[:], mk_d)
            if uniform_bias is None:
                nc.vector.tensor_scalar_add(mk_sb[:], mk_sb[:], -PSHIFT)
            ident = cpool.tile([128, 128], bf16)
            make_identity(nc, ident[:])
            ubias_sb = cpool.tile([128, 1], fp32)
            if uniform_bias is not None:
                nc.gpsimd.memset(ubias_sb[:], uniform_bias)

            qkvs = {}
            vps = {}
            otns = {}
            q_ready = set()

            def emit_vp(b, h, j):
                """One v' tile [128 keys, 64 v + ones] via PE transpose.
                Producer -> drain emitted atomically (no yield in between):
                a yield there lets other generators insert engine-queue ops
                between a psum ring slot's producer and its freeing consumer,
                which can deadlock the in-order queues."""
                vT = qkvs[b][2]
                vtr_ps = ps_a.tile([128, 64], bf16, tag="a", name="vtr_ps")
                nc.tensor.transpose(
                    vtr_ps[:],
                    vT[h * 64:(h + 1) * 64, j * 128:(j + 1) * 128],
                    ident[h * 64:(h + 1) * 64, h * 64:(h + 1) * 64])
                vp = vppool.tile([128, 65], bf16, tag="vp",
                                 name=f"vp_{b}_{h}_{j}")
                nc.vector.tensor_copy(vp[:, 0:64], vtr_ps[:])
                nc.gpsimd.memset(vp[:, 64:65], 1.0)
                vps[(b, h, j)] = vp
                yield 0

            def gen_a(b, head_only=False, tail_only=False):
                """Stage A for batch b, output-major order: all kT tiles,
                then vT (+v'), then qT(t0) [head]; qT(t1..t3) [tail].
                Yields 1 after each matmul, 0 after other units."""
                if not tail_only:
                    qT = qkvpool.tile([128, S], bf16, tag="qT", name=f"qT_{b}")
                    kT = qkvpool.tile([128, S], bf16, tag="kT", name=f"kT_{b}")
                    vT = qkvpool.tile([128, S], bf16, tag="vT", name=f"vT_{b}")
                    xbfs = []
                    for t in range(NQT):
                        xts = []
                        for c in range(DC):
                            xt = xbfpool.tile([128, QT], bf16, tag="xt",
                                              name="xt")
                            # b0: alternate queues (two DMA engines in
                            # parallel, scalar idle pre-attention); b1 loads
                            # during attn(b0) when scalar runs exp -> sync
                            eng = nc.sync if (b == 1 or c % 2 == 0) \
                                else nc.scalar
                            eng.dma_start(
                                xt[:], xt_d[c * 128:(c + 1) * 128,
                                            b * S + t * QT:
                                            b * S + (t + 1) * QT])
                            xts.append(xt)
                        xbfs.append(xts)
                    qkvs[b] = (qT, kT, vT, xbfs)
                qT, kT, vT, xbfs = qkvs[b]

                def chain(dst, w_sb, b_sb, t, tg):
                    a_ps = ps_a.tile([128, QT], fp32, tag="a",
                                     name=f"a_ps_{tg}")
                    for c in range(DC):
                        nc.tensor.matmul(a_ps[:], w_sb[:, c, :],
                                         xbfs[t][c][:],
                                         start=(c == 0), stop=(c == DC - 1))
                        if c < DC - 1:
                            yield 1
                    # drain immediately after the last matmul (see emit_vp)
                    nc.vector.tensor_scalar_add(
                        dst[:, t * QT:(t + 1) * QT], a_ps[:], b_sb[:])
                    yield 1

                def vvp(t):
                    yield from chain(vT, wv_sb, bv_sb, t, "v")
                    for j in range(4 * t, 4 * t + 4):
                        for h in range(HPC):
                            yield from emit_vp(b, h, j)

                if not tail_only:
                    for t in range(NQT):
                        yield from chain(kT, wk_sb, bk_sb, t, "k")
                    yield from vvp(0)
                    yield from chain(qT, wq_sb, bq_sb, 0, "q")
                    q_ready.add((b, 0))
                if not head_only:
                    for t in range(1, NQT):
                        yield from vvp(t)
                        yield from chain(qT, wq_sb, bq_sb, t, "q")
                        q_ready.add((b, t))

            def gen_c(b, rows=None):
                """Stage C for batch b: partial out-projection, bf16 out."""
                otn = otns[b]
                for r in (range(S // 128) if rows is None else rows):
                    co = coutpool.tile([128, D], bf16, tag="co")
                    for n in range(D // QT):
                        c_ps = ps_a.tile([128, QT], fp32, tag="a", name="c_ps")
                        nc.tensor.matmul(c_ps[:],
                                         otn[:, r * 128:(r + 1) * 128],
                                         wp_sb[:, n * QT:(n + 1) * QT],
                                         start=True, stop=True)
                        # drain immediately (see emit_vp)
                        nc.vector.tensor_copy(co[:, n * QT:(n + 1) * QT],
                                              c_ps[:])
                        yield 1
                    nc.sync.dma_start(
                        out_d[b * S + r * 128: b * S + (r + 1) * 128, :],
                        co[:])
                    yield 0

            def pull(side, extra=0):
                for e in side:
                    g, quota = e
                    for _ in range(quota + extra):
                        try:
                            next(g)
                        except StopIteration:
                            break

            def emit_attention(b, side, post_t_side=None, provider=None):
                """provider: generator force-pulled so that qT tiles and v'
                tiles are EMITTED before the attention ops that consume them
                (the in-order PE queue would otherwise deadlock: a consumer
                ahead of its producer in the same queue)."""
                """Attention for batch b. `side` is a list of
                [generator, quota] entries; after each keyblock-pair,
                `quota` units are pulled from each live generator (fills
                the PE while ScalarE runs exp). post_t_side(t) may return
                an entry to append after query tile t's normalization."""
                qT, kT, vT = qkvs[b][:3]
                otn = otnpool.tile([128, S], bf16, tag="otn", name=f"otn_{b}")
                otns[b] = otn
                ot_ps = {}

                def norm(t, h):
                    # reciprocal_approx_fast can't read PSUM (garbage
                    # output when fed ot_ps directly) -- stage via SBUF
                    ll = smpool.tile([1, QT], fp32, tag="ll")
                    nc.vector.tensor_copy(ll[:], ot_ps[(t, h)][64:65, :])
                    rc = smpool.tile([1, QT], fp32, tag="rc")
                    nc.vector.reciprocal_approx_fast(rc[:], ll[:])
                    bc = smpool.tile([64, QT], fp32, tag="bc")
                    nc.gpsimd.partition_broadcast(bc[:], rc[:])
                    if h == 0:
                        nc.vector.tensor_mul(
                            otn[0:64, t * QT:(t + 1) * QT],
                            ot_ps[(t, h)][0:64, :], bc[:])
                    else:
                        hb = smpool.tile([64, QT], bf16, tag="hb")
                        nc.vector.tensor_mul(hb[:], ot_ps[(t, h)][0:64, :],
                                             bc[:])
                        nc.gpsimd.dma_start(
                            otn[64:128, t * QT:(t + 1) * QT], hb[:])

                items = [(t, jj) for t in range(NQT) for jj in range(KB // 2)]
                pend = None  # (t, jj, pt2s) with PV not yet emitted

                def emit_pv(t, jj, pt2s):
                    while provider is not None and \
                            (b, HPC - 1, 2 * jj + 1) not in vps:
                        next(provider)
                    for h in range(HPC):
                        if jj == 0:
                            ot_ps[(t, h)] = ps_ot.tile(
                                [65, QT], fp32, tag="ot",
                                name=f"ot_ps_{b}_{t}_{h}")
                        nc.tensor.matmul(ot_ps[(t, h)][:],
                                         vps[(b, h, 2 * jj)][:],
                                         pt2s[h][:, 0, :],
                                         start=(jj == 0), stop=False)
                        nc.tensor.matmul(ot_ps[(t, h)][:],
                                         vps[(b, h, 2 * jj + 1)][:],
                                         pt2s[h][:, 1, :],
                                         start=False,
                                         stop=(jj == KB // 2 - 1))
                    if jj == KB // 2 - 1:
                        for h in range(HPC):
                            norm(t, h)
                        pull(side, extra=1)
                        if post_t_side is not None:
                            g = post_t_side(t)
                            if g is not None:
                                side.append(g)

                for (t, jj) in items:
                    while provider is not None and (b, t) not in q_ready:
                        next(provider)
                    j0, j1 = 2 * jj, 2 * jj + 1
                    st2s = [ps_st.tile([128, 2, QT], fp32, tag="st",
                                       name=f"st_{h}")
                            for h in range(HPC)]
                    # both heads' K=64 score matmuls issued as an
                    # atomic pair: they occupy disjoint PE row-groups
                    # (rows 0-63 / 64-127) and run concurrently.
                    for ji, jx in ((0, j0), (1, j1)):
                        for h in range(HPC):
                            hs = slice(h * 64, (h + 1) * 64)
                            nc.tensor.matmul(
                                st2s[h][:, ji, :],
                                kT[hs, jx * 128:(jx + 1) * 128],
                                qT[hs, t * QT:(t + 1) * QT],
                                start=True, stop=True)
                    pt2s = []
                    for h in range(HPC):
                        st2 = st2s[h]
                        pt2 = ptpool.tile([128, 2, QT], bf16, tag="pt",
                                          name=f"pt_{h}")
                        if uniform_bias is not None:
                            nc.scalar.activation(pt2[:], st2[:], EXP,
                                                 bias=ubias_sb[:],
                                                 scale=0.125)
                        else:
                            for ji, jx in ((0, j0), (1, j1)):
                                nc.scalar.activation(
                                    pt2[:, ji, :], st2[:, ji, :], EXP,
                                    bias=mk_sb[:, b * KB + jx:
                                               b * KB + jx + 1],
                                    scale=0.125)
                        pt2s.append(pt2)
                    # side work + previous pair's PV: the PE never waits
                    # on the exp it just issued
                    pull(side)
                    if pend is not None:
                        emit_pv(*pend)
                    pend = (t, jj, pt2s)
                emit_pv(*pend)

            def drain(gens):
                for g in gens:
                    for _ in g:
                        pass

            # batch 0: k/v/v'/q(t0) standalone; q(t1..3) + A(b1) fill the
            # exp-latency gaps of attn(b0); C(b0) rows released per-t
            drain([gen_a(0, head_only=True)])
            a0_tail = gen_a(0, tail_only=True)
            a1 = gen_a(1)
            side_b0 = [[a0_tail, 3], [a1, 6]]

            def post_t0(t):
                if t == 0:
                    return None
                return [gen_c(0, rows=range((t - 1) * NQT, t * NQT)), 1]

            emit_attention(0, side_b0, post_t_side=post_t0,
                           provider=a0_tail)
            drain([a0_tail])
            # attn(b1): a1 remainder + remaining C(b0) + per-t C(b1);
            # no standalone drain blob between the attention phases
            side_b1 = [[a1, 3]] + [[e[0], 3] for e in side_b0[2:]]
            side_b1.append([gen_c(0, rows=range(3 * NQT, S // 128)), 3])

            def post_t1(t):
                return [gen_c(1, rows=range(t * NQT, (t + 1) * NQT)), 3]

            emit_attention(1, side_b1, post_t_side=post_t1, provider=a1)
            drain([a1] + [e[0] for e in side_b1])

    nc.compile()
    return nc


def _prep_inputs(x, attention_mask, w_attn, b_attn, w_proj):
    xT = np.ascontiguousarray(
        np.asarray(x, dtype=np.float32).reshape(BS, D).T).astype(BF16)
    maskt = np.ascontiguousarray(
        np.asarray(attention_mask, dtype=np.float32)
        .reshape(B, KB, 128).transpose(2, 0, 1).reshape(128, B * KB))
    w_attn = np.asarray(w_attn, dtype=np.float32)
    b_attn = np.asarray(b_attn, dtype=np.float32)
    w_proj = np.asarray(w_proj, dtype=np.float32)
    in_maps = []
    for c in range(NCORES):
        lo, hi = 2 * c * HD, (2 * c + 2) * HD
        in_maps.append({
            "xt": xT,
            "wq": np.ascontiguousarray(w_attn[:, lo:hi]).astype(BF16),
            "wk": np.ascontiguousarray(w_attn[:, D + lo: D + hi]).astype(BF16),
            "wv": np.ascontiguousarray(
                w_attn[:, 2 * D + lo: 2 * D + hi]).astype(BF16),
            "bq": np.ascontiguousarray(b_attn[lo:hi].reshape(128, 1)),
            "bk": np.ascontiguousarray(b_attn[D + lo: D + hi].reshape(128, 1)),
            "bv": np.ascontiguousarray(
                b_attn[2 * D + lo: 2 * D + hi].reshape(128, 1)),
            "wp": np.ascontiguousarray(w_proj[lo:hi, :]).astype(BF16),
            "maskt": maskt,
        })
    return in_maps


def _run(in_maps, trace=False, tmpdir=None):
    from concourse import bass_utils
    mk = in_maps[0]["maskt"]
    uniform = float(mk.flat[0]) if np.all(mk == mk.flat[0]) else None
    ub = None if uniform is None else uniform - PSHIFT
    key = ("nc", ub)
    if key not in _cache:
        _cache[key] = _build(ub)
    return bass_utils.run_bass_kernel_spmd(
        _cache[key], in_maps, core_ids=list(range(NCORES)),
        trace=trace, tmpdir=tmpdir)


def kernel(x, attention_mask, w_attn, b_attn, w_proj, b_proj):
    in_maps = _prep_inputs(x, attention_mask, w_attn, b_attn, w_proj)
    res = _run(in_maps)
    out = np.zeros((BS, D), dtype=np.float32)
    for c in range(NCORES):
        out += res.results[c]["out"].astype(np.float32)
    out += np.asarray(b_proj, dtype=np.float32)[None, :]
    return out.reshape(B, S, D)


# revision 37
# speedup vs baseline: 1.2374x; 1.0039x over previous
"""Trainium2 Bass kernel for nn_Attention (B=2, S=2048, D=1024, H=16).

Sharding: tensor-parallel over heads. Each of the 8 cores owns 2 heads
(both batches): it computes q,k,v projections for its head columns, full
attention for its 4 (batch, head) pairs, and a partial output projection
(contraction over its 128 head-output columns). The host sums the 8
bf16 partials and adds b_proj.

Notes:
 - All matmuls bf16. fp8 (DoubleRow) was evaluated and rejected: with
   random weights the attention output is a near-uniform average over
   ~750 keys, so per-score quantization noise lands ~0.6x in the final
   output (fp8 q/k => 5% rel err, fp8 p => 2%). Only exact-math wins
   remain.
 - Both heads' K=64 score matmuls are issued as a pair occupying
   disjoint PE row-groups; they run concurrently.
 - Output partials are written bf16 (halves the out DMA).
 - x is loaded with one 3D DMA per (batch, query-tile) slab.
 - Side work (next batch's projections, out-projection rows) is pulled
   between exp and PV so the PE covers the ScalarE exp latency.
"""

import sys

sys.path.insert(0, "/opt/trn_rl_repo")

import numpy as np
import ml_dtypes

B, S, D, H, HD = 2, 2048, 1024, 16, 64
NCORES = 8
HPC = H // NCORES  # heads per core = 2
BS = B * S  # 4096
KB = S // 128  # key blocks per batch = 16
QT = 512  # query tile
NQT = S // QT  # query tiles per batch = 4
DC = D // 128  # contraction chunks = 8
PSHIFT = 1.0  # exp(s/8 + mask - PSHIFT); softmax-invariant shift

BF16 = ml_dtypes.bfloat16

_cache = {}


def _build(uniform_bias):
    import concourse.bass as bass
    import concourse.mybir as mybir
    import concourse.tile as tile
    from concourse import bacc
    from concourse.masks import make_identity

    fp32 = mybir.dt.float32
    bf16 = mybir.dt.bfloat16
    EXP = mybir.ActivationFunctionType.Exp

    nc = bacc.Bacc("TRN2", target_bir_lowering=False, debug=False,
                   num_devices=NCORES)

    xt_d = nc.dram_tensor("xt", [D, BS], bf16, kind="ExternalInput").ap()
    wq_d = nc.dram_tensor("wq", [D, 128], bf16, kind="ExternalInput").ap()
    wk_d = nc.dram_tensor("wk", [D, 128], bf16, kind="ExternalInput").ap()
    wv_d = nc.dram_tensor("wv", [D, 128], bf16, kind="ExternalInput").ap()
    bq_d = nc.dram_tensor("bq", [128, 1], fp32, kind="ExternalInput").ap()
    bk_d = nc.dram_tensor("bk", [128, 1], fp32, kind="ExternalInput").ap()
    bv_d = nc.dram_tensor("bv", [128, 1], fp32, kind="ExternalInput").ap()
    wp_d = nc.dram_tensor("wp", [128, D], bf16, kind="ExternalInput").ap()
    mk_d = nc.dram_tensor("maskt", [128, B * KB], fp32, kind="ExternalInput").ap()
    out_d = nc.dram_tensor("out", [BS, D], bf16, kind="ExternalOutput").ap()

    with tile.TileContext(nc) as tc:
        with (
            tc.tile_pool(name="const", bufs=1) as cpool,
            tc.tile_pool(name="xbf", bufs=64) as xbfpool,
            tc.tile_pool(name="qkv", bufs=2) as qkvpool,
            tc.tile_pool(name="vp", bufs=2 * HPC * KB) as vppool,
            tc.tile_pool(name="pt", bufs=12) as ptpool,
            tc.tile_pool(name="otn", bufs=2) as otnpool,
            tc.tile_pool(name="small", bufs=4) as smpool,
            tc.tile_pool(name="cout", bufs=3) as coutpool,
            tc.tile_pool(name="ps_a", bufs=2, space="PSUM") as ps_a,
            tc.tile_pool(name="ps_st", bufs=2, space="PSUM") as ps_st,
            tc.tile_pool(name="ps_ot", bufs=2, space="PSUM") as ps_ot,
        ):
            # ---- constants ----
            wq_sb = cpool.tile([128, DC, 128], bf16)
            wk_sb = cpool.tile([128, DC, 128], bf16)
            wv_sb = cpool.tile([128, DC, 128], bf16)
            for w_sb, w_d in ((wq_sb, wq_d), (wk_sb, wk_d), (wv_sb, wv_d)):
                nc.gpsimd.dma_start(w_sb[:], w_d.rearrange("(c p) m -> p c m", p=128))
            wp_sb = cpool.tile([128, D], bf16)
            nc.gpsimd.dma_start(wp_sb[:], wp_d)
            bq_sb = cpool.tile([128, 1], fp32)
            bk_sb = cpool.tile([128, 1], fp32)
            bv_sb = cpool.tile([128, 1], fp32)
            for b_sb, b_d in ((bq_sb, bq_d), (bk_sb, bk_d), (bv_sb, bv_d)):
                nc.gpsimd.dma_start(b_sb[:], b_d)
            mk_sb = cpool.tile([128, B * KB], fp32)
            nc.gpsimd.dma_start(mk_sb[:], mk_d)
            if uniform_bias is None:
                nc.vector.tensor_scalar_add(mk_sb[:], mk_sb[:], -PSHIFT)
            ident = cpool.tile([128, 128], bf16)
            make_identity(nc, ident[:])
            ubias_sb = cpool.tile([128, 1], fp32)
            if uniform_bias is not None:
                nc.gpsimd.memset(ubias_sb[:], uniform_bias)

            qkvs = {}
            vps = {}
            otns = {}
            q_ready = set()
            k_done = {0: 0, 1: 0}

            def emit_vp(b, h, j):
                """One v' tile [128 keys, 64 v + ones] via PE transpose.
                Producer -> drain emitted atomically (no yield in between):
                a yield there lets other generators insert engine-queue ops
                between a psum ring slot's producer and its freeing consumer,
                which can deadlock the in-order queues."""
                vT = qkvs[b][2]
                vtr_ps = ps_a.tile([128, 64], bf16, tag="a", name="vtr_ps")
                nc.tensor.transpose(
                    vtr_ps[:],
                    vT[h * 64:(h + 1) * 64, j * 128:(j + 1) * 128],
                    ident[h * 64:(h + 1) * 64, h * 64:(h + 1) * 64])
                vp = vppool.tile([128, 65], bf16, tag="vp",
                                 name=f"vp_{b}_{h}_{j}")
                nc.vector.tensor_copy(vp[:, 0:64], vtr_ps[:])
                nc.gpsimd.memset(vp[:, 64:65], 1.0)
                vps[(b, h, j)] = vp
                yield 0

            def gen_a(b, head_only=False, tail_only=False):
                """Stage A for batch b, output-major order: all kT tiles,
                then vT (+v'), then qT(t0) [head]; qT(t1..t3) [tail].
                Yields 1 after each matmul, 0 after other units."""
                if not tail_only:
                    qT = qkvpool.tile([128, S], bf16, tag="qT", name=f"qT_{b}")
                    kT = qkvpool.tile([128, S], bf16, tag="kT", name=f"kT_{b}")
                    vT = qkvpool.tile([128, S], bf16, tag="vT", name=f"vT_{b}")
                    xbfs = []
                    for t in range(NQT):
                        xts = []
                        for c in range(DC):
                            xt = xbfpool.tile([128, QT], bf16, tag="xt",
                                              name="xt")
                            # b0: alternate queues (two DMA engines in
                            # parallel, scalar idle pre-attention); b1 loads
                            # during attn(b0) when scalar runs exp -> sync
                            eng = nc.sync if (b == 1 or c % 2 == 0) \
                                else nc.scalar
                            eng.dma_start(
                                xt[:], xt_d[c * 128:(c + 1) * 128,
                                            b * S + t * QT:
                                            b * S + (t + 1) * QT])
                            xts.append(xt)
                        xbfs.append(xts)
                    qkvs[b] = (qT, kT, vT, xbfs)
                qT, kT, vT, xbfs = qkvs[b]

                def chain(dst, w_sb, b_sb, t, tg):
                    a_ps = ps_a.tile([128, QT], fp32, tag="a",
                                     name=f"a_ps_{tg}")
                    for c in range(DC):
                        nc.tensor.matmul(a_ps[:], w_sb[:, c, :],
                                         xbfs[t][c][:],
                                         start=(c == 0), stop=(c == DC - 1))
                        if c < DC - 1:
                            yield 1
                    # drain immediately after the last matmul (see emit_vp)
                    nc.vector.tensor_scalar_add(
                        dst[:, t * QT:(t + 1) * QT], a_ps[:], b_sb[:])
                    yield 1

                def vvp(t):
                    yield from chain(vT, wv_sb, bv_sb, t, "v")
                    for j in range(4 * t, 4 * t + 4):
                        for h in range(HPC):
                            yield from emit_vp(b, h, j)

                if not tail_only:
                    yield from chain(kT, wk_sb, bk_sb, 0, "k")
                    k_done[b] = 1
                    yield from vvp(0)
                    yield from chain(qT, wq_sb, bq_sb, 0, "q")
                    q_ready.add((b, 0))
                if not head_only:
                    for t in range(1, NQT):
                        yield from chain(kT, wk_sb, bk_sb, t, "k")
                        k_done[b] = t + 1
                    for t in range(1, NQT):
                        yield from vvp(t)
                        yield from chain(qT, wq_sb, bq_sb, t, "q")
                        q_ready.add((b, t))

            def gen_c(b, rows=None):
                """Stage C for batch b: partial out-projection, bf16 out."""
                otn = otns[b]
                for r in (range(S // 128) if rows is None else rows):
                    co = coutpool.tile([128, D], bf16, tag="co")
                    for n in range(D // QT):
                        c_ps = ps_a.tile([128, QT], fp32, tag="a", name="c_ps")
                        nc.tensor.matmul(c_ps[:],
                                         otn[:, r * 128:(r + 1) * 128],
                                         wp_sb[:, n * QT:(n + 1) * QT],
                                         start=True, stop=True)
                        # drain immediately (see emit_vp)
                        nc.vector.tensor_copy(co[:, n * QT:(n + 1) * QT],
                                              c_ps[:])
                        yield 1
                    nc.sync.dma_start(
                        out_d[b * S + r * 128: b * S + (r + 1) * 128, :],
                        co[:])
                    yield 0

            def pull(side, extra=0):
                for e in side:
                    g, quota = e
                    for _ in range(quota + extra):
                        try:
                            next(g)
                        except StopIteration:
                            break

            def emit_attention(b, side, post_t_side=None, provider=None):
                """provider: generator force-pulled so that qT tiles and v'
                tiles are EMITTED before the attention ops that consume them
                (the in-order PE queue would otherwise deadlock: a consumer
                ahead of its producer in the same queue)."""
                """Attention for batch b. `side` is a list of
                [generator, quota] entries; after each keyblock-pair,
                `quota` units are pulled from each live generator (fills
                the PE while ScalarE runs exp). post_t_side(t) may return
                an entry to append after query tile t's normalization."""
                qT, kT, vT = qkvs[b][:3]
                otn = otnpool.tile([128, S], bf16, tag="otn", name=f"otn_{b}")
                otns[b] = otn
                ot_ps = {}

                def norm(t, h):
                    # reciprocal_approx_fast can't read PSUM (garbage
                    # output when fed ot_ps directly) -- stage via SBUF
                    ll = smpool.tile([1, QT], fp32, tag="ll")
                    nc.vector.tensor_copy(ll[:], ot_ps[(t, h)][64:65, :])
                    rc = smpool.tile([1, QT], fp32, tag="rc")
                    nc.vector.reciprocal_approx_fast(rc[:], ll[:])
                    bc = smpool.tile([64, QT], fp32, tag="bc")
                    nc.gpsimd.partition_broadcast(bc[:], rc[:])
                    if h == 0:
                        nc.vector.tensor_mul(
                            otn[0:64, t * QT:(t + 1) * QT],
                            ot_ps[(t, h)][0:64, :], bc[:])
                    else:
                        hb = smpool.tile([64, QT], bf16, tag="hb")
                        nc.vector.tensor_mul(hb[:], ot_ps[(t, h)][0:64, :],
                                             bc[:])
                        nc.gpsimd.dma_start(
                            otn[64:128, t * QT:(t + 1) * QT], hb[:])

                items = [(t, jj) for t in range(NQT) for jj in range(KB // 2)]
                pend = []  # [(t, jj, pt2s)] with PV not yet emitted; depth 2
                # hides the norm->ot_ps-ring latency at query-tile boundaries

                def emit_pv(t, jj, pt2s):
                    while provider is not None and \
                            (b, HPC - 1, 2 * jj + 1) not in vps:
                        next(provider)
                    for h in range(HPC):
                        if jj == 0:
                            ot_ps[(t, h)] = ps_ot.tile(
                                [65, QT], fp32, tag="ot",
                                name=f"ot_ps_{b}_{t}_{h}")
                        nc.tensor.matmul(ot_ps[(t, h)][:],
                                         vps[(b, h, 2 * jj)][:],
                                         pt2s[h][:, 0, :],
                                         start=(jj == 0), stop=False)
                        nc.tensor.matmul(ot_ps[(t, h)][:],
                                         vps[(b, h, 2 * jj + 1)][:],
                                         pt2s[h][:, 1, :],
                                         start=False,
                                         stop=(jj == KB // 2 - 1))
                    if jj == KB // 2 - 1:
                        for h in range(HPC):
                            norm(t, h)
                        pull(side, extra=1)
                        if post_t_side is not None:
                            g = post_t_side(t)
                            if g is not None:
                                side.append(g)

                for (t, jj) in items:
                    kneed = min(NQT, (2 * jj + 1) // (KB // NQT) + 1)
                    while provider is not None and \
                            ((b, t) not in q_ready or k_done[b] < kneed):
                        next(provider)
                    j0, j1 = 2 * jj, 2 * jj + 1
                    st2s = [ps_st.tile([128, 2, QT], fp32, tag="st",
                                       name=f"st_{h}")
                            for h in range(HPC)]
                    # both heads' K=64 score matmuls issued as an
                    # atomic pair: they occupy disjoint PE row-groups
                    # (rows 0-63 / 64-127) and run concurrently.
                    for ji, jx in ((0, j0), (1, j1)):
                        for h in range(HPC):
                            hs = slice(h * 64, (h + 1) * 64)
                            nc.tensor.matmul(
                                st2s[h][:, ji, :],
                                kT[hs, jx * 128:(jx + 1) * 128],
                                qT[hs, t * QT:(t + 1) * QT],
                                start=True, stop=True)
                    pt2s = []
                    for h in range(HPC):
                        st2 = st2s[h]
                        pt2 = ptpool.tile([128, 2, QT], bf16, tag="pt",
                                          name=f"pt_{h}")
                        if uniform_bias is not None:
                            nc.scalar.activation(pt2[:], st2[:], EXP,
                                                 bias=ubias_sb[:],
                                                 scale=0.125)
                        else:
                            for ji, jx in ((0, j0), (1, j1)):
                                nc.scalar.activation(
                                    pt2[:, ji, :], st2[:, ji, :], EXP,
                                    bias=mk_sb[:, b * KB + jx:
                                               b * KB + jx + 1],
                                    scale=0.125)
                        pt2s.append(pt2)
                    # side work + a lagged PV: the PE never waits on the
                    # exp it just issued
                    pull(side)
                    pend.append((t, jj, pt2s))
                    if len(pend) > 2:
                        emit_pv(*pend.pop(0))
                for p in pend:
                    emit_pv(*p)

            def drain(gens):
                for g in gens:
                    for _ in g:
                        pass

            # batch 0: k/v/v'/q(t0) standalone; q(t1..3) + A(b1) fill the
            # exp-latency gaps of attn(b0); C(b0) rows released per-t
            drain([gen_a(0, head_only=True)])
            a0_tail = gen_a(0, tail_only=True)
            a1 = gen_a(1)
            side_b0 = [[a0_tail, 3], [a1, 6]]

            def post_t0(t):
                if t == 0:
                    return None
                return [gen_c(0, rows=range((t - 1) * NQT, t * NQT)), 1]

            emit_attention(0, side_b0, post_t_side=post_t0,
                           provider=a0_tail)
            drain([a0_tail])
            # attn(b1): a1 remainder + remaining C(b0) + per-t C(b1);
            # no standalone drain blob between the attention phases
            side_b1 = [[a1, 3]] + [[e[0], 3] for e in side_b0[2:]]
            side_b1.append([gen_c(0, rows=range(3 * NQT, S // 128)), 3])

            def post_t1(t):
                return [gen_c(1, rows=range(t * NQT, (t + 1) * NQT)), 4]

            emit_attention(1, side_b1, post_t_side=post_t1, provider=a1)
            drain([a1] + [e[0] for e in side_b1])

    nc.compile()
    return nc


def _prep_inputs(x, attention_mask, w_attn, b_attn, w_proj):
    xT = np.ascontiguousarray(
        np.asarray(x, dtype=np.float32).reshape(BS, D).T).astype(BF16)
    maskt = np.ascontiguousarray(
        np.asarray(attention_mask, dtype=np.float32)
        .reshape(B, KB, 128).transpose(2, 0, 1).reshape(128, B * KB))
    w_attn = np.asarray(w_attn, dtype=np.float32)
    b_attn = np.asarray(b_attn, dtype=np.float32)
    w_proj = np.asarray(w_proj, dtype=np.float32)
    in_maps = []
    for c in range(NCORES):
        lo, hi = 2 * c * HD, (2 * c + 2) * HD
        in_maps.append({
            "xt": xT,
            "wq": np.ascontiguousarray(w_attn[:, lo:hi]).astype(BF16),
            "wk": np.ascontiguousarray(w_attn[:, D + lo: D + hi]).astype(BF16),
            "wv": np.ascontiguousarray(
                w_attn[:, 2 * D + lo: 2 * D + hi]).astype(BF16),
            "bq": np.ascontiguousarray(b_attn[lo:hi].reshape(128, 1)),
            "bk": np.ascontiguousarray(b_attn[D + lo: D + hi].reshape(128, 1)),
            "bv": np.ascontiguousarray(
                b_attn[2 * D + lo: 2 * D + hi].reshape(128, 1)),
            "wp": np.ascontiguousarray(w_proj[lo:hi, :]).astype(BF16),
            "maskt": maskt,
        })
    return in_maps


def _run(in_maps, trace=False, tmpdir=None):
    from concourse import bass_utils
    mk = in_maps[0]["maskt"]
    uniform = float(mk.flat[0]) if np.all(mk == mk.flat[0]) else None
    ub = None if uniform is None else uniform - PSHIFT
    key = ("nc", ub)
    if key not in _cache:
        _cache[key] = _build(ub)
    return bass_utils.run_bass_kernel_spmd(
        _cache[key], in_maps, core_ids=list(range(NCORES)),
        trace=trace, tmpdir=tmpdir)


def kernel(x, attention_mask, w_attn, b_attn, w_proj, b_proj):
    in_maps = _prep_inputs(x, attention_mask, w_attn, b_attn, w_proj)
    res = _run(in_maps)
    out = np.zeros((BS, D), dtype=np.float32)
    for c in range(NCORES):
        out += res.results[c]["out"].astype(np.float32)
    out += np.asarray(b_proj, dtype=np.float32)[None, :]
    return out.reshape(B, S, D)


# revision 38
# speedup vs baseline: 1.2460x; 1.0070x over previous
"""Trainium2 Bass kernel for nn_Attention (B=2, S=2048, D=1024, H=16).

Sharding: tensor-parallel over heads. Each of the 8 cores owns 2 heads
(both batches): it computes q,k,v projections for its head columns, full
attention for its 4 (batch, head) pairs, and a partial output projection
(contraction over its 128 head-output columns). The host sums the 8
bf16 partials and adds b_proj.

Notes:
 - All matmuls bf16. fp8 (DoubleRow) was evaluated and rejected: with
   random weights the attention output is a near-uniform average over
   ~750 keys, so per-score quantization noise lands ~0.6x in the final
   output (fp8 q/k => 5% rel err, fp8 p => 2%). Only exact-math wins
   remain.
 - Both heads' K=64 score matmuls are issued as a pair occupying
   disjoint PE row-groups; they run concurrently.
 - Output partials are written bf16 (halves the out DMA).
 - x is loaded with one 3D DMA per (batch, query-tile) slab.
 - Side work (next batch's projections, out-projection rows) is pulled
   between exp and PV so the PE covers the ScalarE exp latency.
"""

import sys

sys.path.insert(0, "/opt/trn_rl_repo")

import numpy as np
import ml_dtypes

B, S, D, H, HD = 2, 2048, 1024, 16, 64
NCORES = 8
HPC = H // NCORES  # heads per core = 2
BS = B * S  # 4096
KB = S // 128  # key blocks per batch = 16
QT = 512  # query tile
NQT = S // QT  # query tiles per batch = 4
DC = D // 128  # contraction chunks = 8
PSHIFT = 1.0  # exp(s/8 + mask - PSHIFT); softmax-invariant shift

BF16 = ml_dtypes.bfloat16

_cache = {}


def _build(uniform_bias):
    import concourse.bass as bass
    import concourse.mybir as mybir
    import concourse.tile as tile
    from concourse import bacc
    from concourse.masks import make_identity

    fp32 = mybir.dt.float32
    bf16 = mybir.dt.bfloat16
    EXP = mybir.ActivationFunctionType.Exp

    nc = bacc.Bacc("TRN2", target_bir_lowering=False, debug=False,
                   num_devices=NCORES)

    xt_d = nc.dram_tensor("xt", [D, BS], bf16, kind="ExternalInput").ap()
    wq_d = nc.dram_tensor("wq", [D, 128], bf16, kind="ExternalInput").ap()
    wk_d = nc.dram_tensor("wk", [D, 128], bf16, kind="ExternalInput").ap()
    wv_d = nc.dram_tensor("wv", [D, 128], bf16, kind="ExternalInput").ap()
    bq_d = nc.dram_tensor("bq", [128, 1], fp32, kind="ExternalInput").ap()
    bk_d = nc.dram_tensor("bk", [128, 1], fp32, kind="ExternalInput").ap()
    bv_d = nc.dram_tensor("bv", [128, 1], fp32, kind="ExternalInput").ap()
    wp_d = nc.dram_tensor("wp", [128, D], bf16, kind="ExternalInput").ap()
    mk_d = nc.dram_tensor("maskt", [128, B * KB], fp32, kind="ExternalInput").ap()
    out_d = nc.dram_tensor("out", [BS, D], bf16, kind="ExternalOutput").ap()

    with tile.TileContext(nc) as tc:
        with (
            tc.tile_pool(name="const", bufs=1) as cpool,
            tc.tile_pool(name="xbf", bufs=64) as xbfpool,
            tc.tile_pool(name="qkv", bufs=2) as qkvpool,
            tc.tile_pool(name="vp", bufs=2 * HPC * KB) as vppool,
            tc.tile_pool(name="pt", bufs=12) as ptpool,
            tc.tile_pool(name="otn", bufs=2) as otnpool,
            tc.tile_pool(name="small", bufs=4) as smpool,
            tc.tile_pool(name="cout", bufs=3) as coutpool,
            tc.tile_pool(name="ps_a", bufs=2, space="PSUM") as ps_a,
            tc.tile_pool(name="ps_st", bufs=2, space="PSUM") as ps_st,
            tc.tile_pool(name="ps_ot", bufs=2, space="PSUM") as ps_ot,
        ):
            # ---- constants ----
            wq_sb = cpool.tile([128, DC, 128], bf16)
            wk_sb = cpool.tile([128, DC, 128], bf16)
            wv_sb = cpool.tile([128, DC, 128], bf16)
            for w_sb, w_d in ((wq_sb, wq_d), (wk_sb, wk_d), (wv_sb, wv_d)):
                nc.gpsimd.dma_start(w_sb[:], w_d.rearrange("(c p) m -> p c m", p=128))
            wp_sb = cpool.tile([128, D], bf16)
            nc.gpsimd.dma_start(wp_sb[:], wp_d)
            bq_sb = cpool.tile([128, 1], fp32)
            bk_sb = cpool.tile([128, 1], fp32)
            bv_sb = cpool.tile([128, 1], fp32)
            for b_sb, b_d in ((bq_sb, bq_d), (bk_sb, bk_d), (bv_sb, bv_d)):
                nc.gpsimd.dma_start(b_sb[:], b_d)
            mk_sb = cpool.tile([128, B * KB], fp32)
            nc.gpsimd.dma_start(mk_sb[:], mk_d)
            if uniform_bias is None:
                nc.vector.tensor_scalar_add(mk_sb[:], mk_sb[:], -PSHIFT)
            ident = cpool.tile([128, 128], bf16)
            make_identity(nc, ident[:])
            ubias_sb = cpool.tile([128, 1], fp32)
            if uniform_bias is not None:
                nc.gpsimd.memset(ubias_sb[:], uniform_bias)

            qkvs = {}
            vps = {}
            otns = {}
            q_ready = set()
            k_done = {0: 0, 1: 0}

            def emit_vp(b, h, j):
                """One v' tile [128 keys, 64 v + ones] via PE transpose.
                Producer -> drain emitted atomically (no yield in between):
                a yield there lets other generators insert engine-queue ops
                between a psum ring slot's producer and its freeing consumer,
                which can deadlock the in-order queues."""
                vT = qkvs[b][2]
                vtr_ps = ps_a.tile([128, 64], bf16, tag="a", name="vtr_ps")
                nc.tensor.transpose(
                    vtr_ps[:],
                    vT[h * 64:(h + 1) * 64, j * 128:(j + 1) * 128],
                    ident[h * 64:(h + 1) * 64, h * 64:(h + 1) * 64])
                vp = vppool.tile([128, 65], bf16, tag="vp",
                                 name=f"vp_{b}_{h}_{j}")
                nc.vector.tensor_copy(vp[:, 0:64], vtr_ps[:])
                nc.gpsimd.memset(vp[:, 64:65], 1.0)
                vps[(b, h, j)] = vp
                yield 0

            def gen_a(b, head_only=False, tail_only=False):
                """Stage A for batch b, output-major order: all kT tiles,
                then vT (+v'), then qT(t0) [head]; qT(t1..t3) [tail].
                Yields 1 after each matmul, 0 after other units."""
                if not tail_only:
                    qT = qkvpool.tile([128, S], bf16, tag="qT", name=f"qT_{b}")
                    kT = qkvpool.tile([128, S], bf16, tag="kT", name=f"kT_{b}")
                    vT = qkvpool.tile([128, S], bf16, tag="vT", name=f"vT_{b}")
                    xbfs = []
                    for t in range(NQT):
                        xts = []
                        for c in range(DC):
                            xt = xbfpool.tile([128, QT], bf16, tag="xt",
                                              name="xt")
                            # b0: alternate queues (two DMA engines in
                            # parallel, scalar idle pre-attention); b1 loads
                            # during attn(b0) when scalar runs exp -> sync
                            eng = nc.sync if (b == 1 or c % 2 == 0) \
                                else nc.scalar
                            eng.dma_start(
                                xt[:], xt_d[c * 128:(c + 1) * 128,
                                            b * S + t * QT:
                                            b * S + (t + 1) * QT])
                            xts.append(xt)
                        xbfs.append(xts)
                    qkvs[b] = (qT, kT, vT, xbfs)
                qT, kT, vT, xbfs = qkvs[b]

                def chain(dst, w_sb, b_sb, t, tg):
                    a_ps = ps_a.tile([128, QT], fp32, tag="a",
                                     name=f"a_ps_{tg}")
                    for c in range(DC):
                        nc.tensor.matmul(a_ps[:], w_sb[:, c, :],
                                         xbfs[t][c][:],
                                         start=(c == 0), stop=(c == DC - 1))
                        if c < DC - 1:
                            yield 1
                    # drain immediately after the last matmul (see emit_vp)
                    nc.vector.tensor_scalar_add(
                        dst[:, t * QT:(t + 1) * QT], a_ps[:], b_sb[:])
                    yield 1

                def vvp(t):
                    yield from chain(vT, wv_sb, bv_sb, t, "v")
                    for j in range(4 * t, 4 * t + 4):
                        for h in range(HPC):
                            yield from emit_vp(b, h, j)

                if not tail_only:
                    yield from chain(kT, wk_sb, bk_sb, 0, "k")
                    k_done[b] = 1
                    yield from vvp(0)
                    yield from chain(qT, wq_sb, bq_sb, 0, "q")
                    q_ready.add((b, 0))
                if not head_only:
                    for t in range(1, NQT):
                        yield from chain(kT, wk_sb, bk_sb, t, "k")
                        k_done[b] = t + 1
                    for t in range(1, NQT):
                        yield from vvp(t)
                        yield from chain(qT, wq_sb, bq_sb, t, "q")
                        q_ready.add((b, t))

            def gen_c(b, rows=None):
                """Stage C for batch b: partial out-projection, bf16 out."""
                otn = otns[b]
                for r in (range(S // 128) if rows is None else rows):
                    co = coutpool.tile([128, D], bf16, tag="co")
                    for n in range(D // QT):
                        c_ps = ps_a.tile([128, QT], fp32, tag="a", name="c_ps")
                        nc.tensor.matmul(c_ps[:],
                                         otn[:, r * 128:(r + 1) * 128],
                                         wp_sb[:, n * QT:(n + 1) * QT],
                                         start=True, stop=True)
                        # drain immediately (see emit_vp)
                        nc.vector.tensor_copy(co[:, n * QT:(n + 1) * QT],
                                              c_ps[:])
                        yield 1
                    nc.sync.dma_start(
                        out_d[b * S + r * 128: b * S + (r + 1) * 128, :],
                        co[:])
                    yield 0

            def pull(side, extra=0):
                for e in side:
                    g, quota = e
                    for _ in range(quota + extra):
                        try:
                            next(g)
                        except StopIteration:
                            break

            def emit_attention(b, side, post_t_side=None, provider=None):
                """provider: generator force-pulled so that qT tiles and v'
                tiles are EMITTED before the attention ops that consume them
                (the in-order PE queue would otherwise deadlock: a consumer
                ahead of its producer in the same queue)."""
                """Attention for batch b. `side` is a list of
                [generator, quota] entries; after each keyblock-pair,
                `quota` units are pulled from each live generator (fills
                the PE while ScalarE runs exp). post_t_side(t) may return
                an entry to append after query tile t's normalization."""
                qT, kT, vT = qkvs[b][:3]
                otn = otnpool.tile([128, S], bf16, tag="otn", name=f"otn_{b}")
                otns[b] = otn
                ot_ps = {}

                def norm(t, h):
                    # reciprocal_approx_fast can't read PSUM (garbage
                    # output when fed ot_ps directly) -- stage via SBUF
                    ll = smpool.tile([1, QT], fp32, tag="ll")
                    nc.vector.tensor_copy(ll[:], ot_ps[(t, h)][64:65, :])
                    rc = smpool.tile([1, QT], fp32, tag="rc")
                    nc.vector.reciprocal_approx_fast(rc[:], ll[:])
                    bc = smpool.tile([64, QT], fp32, tag="bc")
                    nc.gpsimd.partition_broadcast(bc[:], rc[:])
                    if h == 0:
                        nc.vector.tensor_mul(
                            otn[0:64, t * QT:(t + 1) * QT],
                            ot_ps[(t, h)][0:64, :], bc[:])
                    else:
                        hb = smpool.tile([64, QT], bf16, tag="hb")
                        nc.vector.tensor_mul(hb[:], ot_ps[(t, h)][0:64, :],
                                             bc[:])
                        nc.gpsimd.dma_start(
                            otn[64:128, t * QT:(t + 1) * QT], hb[:])

                items = [(t, jj) for t in range(NQT) for jj in range(KB // 2)]
                pend = []  # [(t, jj, pt2s)] with PV not yet emitted; depth 2
                # hides the norm->ot_ps-ring latency at query-tile boundaries

                def emit_pv(t, jj, pt2s):
                    while provider is not None and \
                            (b, HPC - 1, 2 * jj + 1) not in vps:
                        next(provider)
                    for h in range(HPC):
                        if jj == 0:
                            ot_ps[(t, h)] = ps_ot.tile(
                                [65, QT], fp32, tag="ot",
                                name=f"ot_ps_{b}_{t}_{h}")
                        nc.tensor.matmul(ot_ps[(t, h)][:],
                                         vps[(b, h, 2 * jj)][:],
                                         pt2s[h][:, 0, :],
                                         start=(jj == 0), stop=False)
                        nc.tensor.matmul(ot_ps[(t, h)][:],
                                         vps[(b, h, 2 * jj + 1)][:],
                                         pt2s[h][:, 1, :],
                                         start=False,
                                         stop=(jj == KB // 2 - 1))
                    if jj == KB // 2 - 1:
                        for h in range(HPC):
                            norm(t, h)
                        pull(side, extra=1)
                        if post_t_side is not None:
                            g = post_t_side(t)
                            if g is not None:
                                side.append(g)

                def force(idx):
                    # make the inputs of item idx+2 available ahead of time:
                    # spreads the provider's bursts across items instead of
                    # stalling the Scalar pipeline right before a consumer
                    ta, ja = items[min(idx + 2, len(items) - 1)]
                    kneed = min(NQT, (2 * ja + 1) // (KB // NQT) + 1)
                    while provider is not None and \
                            ((b, ta) not in q_ready or k_done[b] < kneed
                             or (b, HPC - 1, 2 * ja + 1) not in vps):
                        try:
                            next(provider)
                        except StopIteration:
                            break

                for idx, (t, jj) in enumerate(items):
                    force(idx)
                    j0, j1 = 2 * jj, 2 * jj + 1
                    st2s = [ps_st.tile([128, 2, QT], fp32, tag="st",
                                       name=f"st_{h}")
                            for h in range(HPC)]
                    # both heads' K=64 score matmuls issued as an
                    # atomic pair: they occupy disjoint PE row-groups
                    # (rows 0-63 / 64-127) and run concurrently.
                    for ji, jx in ((0, j0), (1, j1)):
                        for h in range(HPC):
                            hs = slice(h * 64, (h + 1) * 64)
                            nc.tensor.matmul(
                                st2s[h][:, ji, :],
                                kT[hs, jx * 128:(jx + 1) * 128],
                                qT[hs, t * QT:(t + 1) * QT],
                                start=True, stop=True)
                    pt2s = []
                    for h in range(HPC):
                        st2 = st2s[h]
                        pt2 = ptpool.tile([128, 2, QT], bf16, tag="pt",
                                          name=f"pt_{h}")
                        if uniform_bias is not None:
                            nc.scalar.activation(pt2[:], st2[:], EXP,
                                                 bias=ubias_sb[:],
                                                 scale=0.125)
                        else:
                            for ji, jx in ((0, j0), (1, j1)):
                                nc.scalar.activation(
                                    pt2[:, ji, :], st2[:, ji, :], EXP,
                                    bias=mk_sb[:, b * KB + jx:
                                               b * KB + jx + 1],
                                    scale=0.125)
                        pt2s.append(pt2)
                    # side work + a lagged PV: the PE never waits on the
                    # exp it just issued
                    pull(side)
                    pend.append((t, jj, pt2s))
                    if len(pend) > 2:
                        emit_pv(*pend.pop(0))
                for p in pend:
                    emit_pv(*p)

            def drain(gens):
                for g in gens:
                    for _ in g:
                        pass

            # batch 0: k/v/v'/q(t0) standalone; q(t1..3) + A(b1) fill the
            # exp-latency gaps of attn(b0); C(b0) rows released per-t
            drain([gen_a(0, head_only=True)])
            a0_tail = gen_a(0, tail_only=True)
            a1 = gen_a(1)
            side_b0 = [[a0_tail, 3], [a1, 6]]

            def post_t0(t):
                if t == 0:
                    return None
                return [gen_c(0, rows=range((t - 1) * NQT, t * NQT)), 1]

            emit_attention(0, side_b0, post_t_side=post_t0,
                           provider=a0_tail)
            drain([a0_tail])
            # attn(b1): a1 remainder + remaining C(b0) + per-t C(b1);
            # no standalone drain blob between the attention phases
            side_b1 = [[a1, 3]] + [[e[0], 3] for e in side_b0[2:]]
            side_b1.append([gen_c(0, rows=range(3 * NQT, S // 128)), 3])

            def post_t1(t):
                return [gen_c(1, rows=range(t * NQT, (t + 1) * NQT)), 4]

            emit_attention(1, side_b1, post_t_side=post_t1, provider=a1)
            drain([a1] + [e[0] for e in side_b1])

    nc.compile()
    return nc


def _prep_inputs(x, attention_mask, w_attn, b_attn, w_proj):
    xT = np.ascontiguousarray(
        np.asarray(x, dtype=np.float32).reshape(BS, D).T).astype(BF16)
    maskt = np.ascontiguousarray(
        np.asarray(attention_mask, dtype=np.float32)
        .reshape(B, KB, 128).transpose(2, 0, 1).reshape(128, B * KB))
    w_attn = np.asarray(w_attn, dtype=np.float32)
    b_attn = np.asarray(b_attn, dtype=np.float32)
    w_proj = np.asarray(w_proj, dtype=np.float32)
    in_maps = []
    for c in range(NCORES):
        lo, hi = 2 * c * HD, (2 * c + 2) * HD
        in_maps.append({
            "xt": xT,
            "wq": np.ascontiguousarray(w_attn[:, lo:hi]).astype(BF16),
            "wk": np.ascontiguousarray(w_attn[:, D + lo: D + hi]).astype(BF16),
            "wv": np.ascontiguousarray(
                w_attn[:, 2 * D + lo: 2 * D + hi]).astype(BF16),
            "bq": np.ascontiguousarray(b_attn[lo:hi].reshape(128, 1)),
            "bk": np.ascontiguousarray(b_attn[D + lo: D + hi].reshape(128, 1)),
            "bv": np.ascontiguousarray(
                b_attn[2 * D + lo: 2 * D + hi].reshape(128, 1)),
            "wp": np.ascontiguousarray(w_proj[lo:hi, :]).astype(BF16),
            "maskt": maskt,
        })
    return in_maps


def _run(in_maps, trace=False, tmpdir=None):
    from concourse import bass_utils
    mk = in_maps[0]["maskt"]
    uniform = float(mk.flat[0]) if np.all(mk == mk.flat[0]) else None
    ub = None if uniform is None else uniform - PSHIFT
    key = ("nc", ub)
    if key not in _cache:
        _cache[key] = _build(ub)
    return bass_utils.run_bass_kernel_spmd(
        _cache[key], in_maps, core_ids=list(range(NCORES)),
        trace=trace, tmpdir=tmpdir)


def kernel(x, attention_mask, w_attn, b_attn, w_proj, b_proj):
    in_maps = _prep_inputs(x, attention_mask, w_attn, b_attn, w_proj)
    res = _run(in_maps)
    out = np.zeros((BS, D), dtype=np.float32)
    for c in range(NCORES):
        out += res.results[c]["out"].astype(np.float32)
    out += np.asarray(b_proj, dtype=np.float32)[None, :]
    return out.reshape(B, S, D)
